# revision 16
# baseline (speedup 1.0000x reference)
"""Trainium2 Bass kernel for the compositional skeleton loss.

loss = mean_b sum_{pairs p, xyz c} | (C @ bones_in)[b,p,c] - (T @ bones_tgt)[b,p,c] |

Reformulated as one matmul per batch row:  delta_row = z_row @ W, where
z_row = [input_row (63), target_row (63)] and W is [126, 630] built from the
signed path-sum matrix C and the endpoint-diff matrix T (block structure over
the 3 xyz channels), followed by abs + total sum, / B.

Device dataflow (per core, pure data parallel over batch across 8 cores):
  - host casts z to bf16 (loss rel err ~1e-6, tolerance 2e-2) padded to
    128 features
  - the DMA XBAR transpose loads z.T straight from DRAM into SBUF
    ([128 features, batch]), so the PE does no transposes and no
    PSUM->SBUF staging copies are needed
  - bf16 matmuls W_chunk.T @ zT fill [126, 1024] PSUM tiles (1 cyc/row
    vs fp32's 4)
  - abs+sum of each PSUM tile is load-balanced across ACT (activation
    with accum_out) and DVE (tensor_reduce X) -- the only two engines
    that can read PSUM on TRN2 (GPSIMD/Pool and DMA cannot)
  - one final DVE reduce -> [126,1] partials -> DRAM; host adds them up

At the cost-model level this is vector-floor-bound: 40960 PSUM columns
per core split across ACT (~0.84 col/ns) + DVE (~0.86 col/ns), with the
PE (bf16 matmuls) at ~60% occupancy and DMA at ~25%.
"""

import numpy as np
from collections import deque
from itertools import combinations

import ml_dtypes

# ---------------------------------------------------------------- constants
NJ = 21
B_FULL = 65536
N_CORES = 8
B_CORE = B_FULL // N_CORES  # 8192
N_GRP = 8                   # batch groups per core
G_ROWS = B_CORE // N_GRP    # 1024 rows per group
N_CCH = 5                   # 630 = 5 x 126 output-column chunks

_JOINTS = ['Ab', 'Chest', 'Head', 'Hip', 'LFArm', 'LFoot', 'LHand', 'LShin',
           'LShoulder', 'LThigh', 'LToe', 'LUArm', 'Neck', 'RFArm', 'RFoot',
           'RHand', 'RShin', 'RShoulder', 'RThigh', 'RToe', 'RUArm']
_PARENTS = {'Ab': 'Hip', 'Chest': 'Ab', 'Head': 'Neck', 'Hip': 'Hip',
            'LFArm': 'LUArm', 'LFoot': 'LShin', 'LHand': 'LFArm',
            'LShin': 'LThigh', 'LShoulder': 'Chest', 'LThigh': 'Hip',
            'LToe': 'LFoot', 'LUArm': 'LShoulder', 'Neck': 'Chest',
            'RFArm': 'RUArm', 'RFoot': 'RShin', 'RHand': 'RFArm',
            'RShin': 'RThigh', 'RShoulder': 'Chest', 'RThigh': 'Hip',
            'RToe': 'RFoot', 'RUArm': 'RShoulder'}


def _build_w():
    idx = {n: i for i, n in enumerate(_JOINTS)}
    par = {idx[k]: idx[v] for k, v in _PARENTS.items()}
    adj = {j: [] for j in range(NJ)}
    for j, p in par.items():
        if j != p:
            adj[j].append(p)
            adj[p].append(j)

    def bfs_path(u, v):
        prev = {u: None}
        q = deque([u])
        while q:
            x = q.popleft()
            if x == v:
                break
            for y in adj[x]:
                if y not in prev:
                    prev[y] = x
                    q.append(y)
        path = [v]
        while prev[path[-1]] is not None:
            path.append(prev[path[-1]])
        return path[::-1]

    pairs = list(combinations(range(NJ), 2))  # 210
    c_np = np.zeros((len(pairs), NJ), np.float32)
    t_np = np.zeros((len(pairs), NJ), np.float32)
    for pi, (u, v) in enumerate(pairs):
        pa = bfs_path(u, v)
        for m in range(len(pa) - 1):
            c_np[pi, pa[m]] += 1.0 if par[pa[m]] == pa[m + 1] else -1.0
        t_np[pi, u] += 1.0
        t_np[pi, v] -= 1.0

    # W[t*63 + j*3 + c, p*3 + c] = C[p,j] (t=0) / -T[p,j] (t=1)
    eye3 = np.eye(3, dtype=np.float32)
    w_in = np.einsum('pj,cd->jcpd', c_np, eye3).reshape(63, 630)
    w_tg = np.einsum('pj,cd->jcpd', -t_np, eye3).reshape(63, 630)
    return np.ascontiguousarray(np.concatenate([w_in, w_tg], axis=0))  # [126, 630]


_W = _build_w()
# padded to 128 contraction rows, bf16 (entries are 0/+-1: exact)
_WP = np.zeros((128, 630), np.float32)
_WP[:126] = _W
_WP_BF16 = _WP.astype(ml_dtypes.bfloat16)

_NC = None


def _build_bass(n_reps=1):
    import concourse.bacc as bacc
    import concourse.mybir as mybir
    import concourse.tile as tile

    f32 = mybir.dt.float32
    bf16 = mybir.dt.bfloat16
    nc = bacc.Bacc("TRN2", target_bir_lowering=False, debug=False)

    z = nc.dram_tensor("z", [B_CORE, 128], bf16, kind="ExternalInput")
    out = nc.dram_tensor("out", [126, 1], f32, kind="ExternalOutput")

    w_dram = nc.inline_tensor(_WP_BF16, name="w_const")

    n_cols = N_GRP * N_CCH  # 40 abs-reduce results per rep

    with tile.TileContext(nc) as tc:
        with (
            tc.tile_pool(name="consts", bufs=1) as consts,
            tc.tile_pool(name="zt", bufs=8) as zt_pool,
            tc.tile_pool(name="accp", bufs=2) as accp,
            tc.tile_pool(name="psumD", bufs=4, space="PSUM") as psumD_pool,
            tc.tile_pool(name="misc", bufs=1) as misc,
        ):
            w_sb = consts.tile([128, 630], bf16)
            nc.sync.dma_start(w_sb[:], w_dram[:])

            # abs dump (never read back); bf16 to halve SBUF write traffic
            scratch = misc.tile([126, 1024], bf16)
            # prime the ACT Abs table so LoadActFuncSet overlaps the
            # initial DMA fill instead of stalling the first abs-reduce
            nc.scalar.activation(scratch[0:1, 0:1], scratch[0:1, 0:1],
                                 mybir.ActivationFunctionType.Abs)

            for rep in range(n_reps):
                acc = accp.tile([126, 2 * n_cols], f32, tag="acc")
                final = accp.tile([126, 1], f32, tag="final")

                # greedy ACT/DVE balance (ns-per-op estimates; ACT =
                # activation + accumulator read). GPSIMD cannot touch
                # PSUM on TRN2, so Pool sits this out.
                eng_t = {"act": 0.0, "dve": 0.0}
                cost = {"act": 1225.0, "dve": 1192.0}
                n_c = 0

                def emit_absred(dps):
                    nonlocal n_c
                    col = acc[:, n_c:n_c + 1]
                    n_c += 1
                    e = min(eng_t, key=lambda k: eng_t[k] + cost[k])
                    eng_t[e] += cost[e]
                    if e == "act":
                        nc.scalar.activation(
                            scratch[:], dps[:],
                            mybir.ActivationFunctionType.Abs,
                            accum_out=col)
                    else:
                        nc.vector.tensor_reduce(
                            col, dps[:],
                            axis=mybir.AxisListType.X,
                            op=mybir.AluOpType.add,
                            apply_absolute_value=True)

                for g in range(N_GRP):
                    zt = zt_pool.tile([128, G_ROWS], bf16)
                    nc.sync.dma_start_transpose(
                        zt[:], z[g * G_ROWS:(g + 1) * G_ROWS, :])

                    for c in range(N_CCH):
                        dps = psumD_pool.tile([126, 1024], f32)
                        nc.tensor.matmul(
                            dps[:, 0:512],
                            w_sb[:, c * 126:(c + 1) * 126], zt[:, 0:512])
                        nc.tensor.matmul(
                            dps[:, 512:1024],
                            w_sb[:, c * 126:(c + 1) * 126], zt[:, 512:1024])
                        emit_absred(dps)

                nc.vector.tensor_reduce(
                    final[:], acc[:, 0:n_c], axis=mybir.AxisListType.X,
                    op=mybir.AluOpType.add)
                nc.sync.dma_start(out[:], final[:])

    nc.compile()
    return nc


def kernel(input, target):
    global _NC
    from concourse.bass_utils import run_bass_kernel_spmd

    if _NC is None:
        _NC = _build_bass()

    inp = np.asarray(input, dtype=np.float32)
    tgt = np.asarray(target, dtype=np.float32)
    assert inp.shape == (B_FULL, NJ * 3) and tgt.shape == (B_FULL, NJ * 3)

    z = np.zeros((B_FULL, 128), dtype=ml_dtypes.bfloat16)
    z[:, 0:63] = inp.astype(ml_dtypes.bfloat16)
    z[:, 63:126] = tgt.astype(ml_dtypes.bfloat16)

    in_maps = []
    for i in range(N_CORES):
        sl = slice(i * B_CORE, (i + 1) * B_CORE)
        in_maps.append({"z": np.ascontiguousarray(z[sl])})

    res = run_bass_kernel_spmd(_NC, in_maps, core_ids=list(range(N_CORES)))
    total = np.float64(0.0)
    for r in res.results:
        total += np.float64(r["out"].astype(np.float64).sum())
    return np.array([total / B_FULL], dtype=np.float32)


# revision 17
# speedup vs baseline: 1.0932x; 1.0932x over previous
"""Trainium2 Bass kernel for the compositional skeleton loss.

loss = mean_b sum_{pairs p, xyz c} | (C @ bones_in)[b,p,c] - (T @ bones_tgt)[b,p,c] |

Reformulated as one matmul per batch row:  delta_row = z_row @ W, where
z_row = [input_row (63), target_row (63)] and W is [126, 630] built from the
signed path-sum matrix C and the endpoint-diff matrix T (block structure over
the 3 xyz channels), followed by abs + total sum, / B.

Device dataflow (per core, pure data parallel over batch across 8 cores):
  - host casts z to bf16 (loss rel err ~1e-6, tolerance 2e-2) padded to
    128 features
  - the DMA XBAR transpose loads z.T straight from DRAM into SBUF
    ([128 features, batch]), so the PE does no transposes and no
    PSUM->SBUF staging copies are needed
  - bf16 matmuls W_chunk.T @ zT fill [126, 1024] PSUM tiles (1 cyc/row
    vs fp32's 4)
  - abs+sum of each PSUM tile is load-balanced across ACT (activation
    with accum_out) and DVE (tensor_reduce X) -- the only two engines
    that can read PSUM on TRN2 (GPSIMD/Pool and DMA cannot)
  - one final DVE reduce -> [126,1] partials -> DRAM; host adds them up

At the cost-model level this is vector-floor-bound: 40960 PSUM columns
per core split across ACT (~0.84 col/ns) + DVE (~0.86 col/ns), with the
PE (bf16 matmuls) at ~60% occupancy and DMA at ~25%.
"""

import numpy as np
from collections import deque
from itertools import combinations

import ml_dtypes

# ---------------------------------------------------------------- constants
NJ = 21
B_FULL = 65536
N_CORES = 8
B_CORE = B_FULL // N_CORES  # 8192
N_GRP = 8                   # batch groups per core
G_ROWS = B_CORE // N_GRP    # 1024 rows per group
N_CCH = 5                   # 630 = 5 x 126 output-column chunks

_JOINTS = ['Ab', 'Chest', 'Head', 'Hip', 'LFArm', 'LFoot', 'LHand', 'LShin',
           'LShoulder', 'LThigh', 'LToe', 'LUArm', 'Neck', 'RFArm', 'RFoot',
           'RHand', 'RShin', 'RShoulder', 'RThigh', 'RToe', 'RUArm']
_PARENTS = {'Ab': 'Hip', 'Chest': 'Ab', 'Head': 'Neck', 'Hip': 'Hip',
            'LFArm': 'LUArm', 'LFoot': 'LShin', 'LHand': 'LFArm',
            'LShin': 'LThigh', 'LShoulder': 'Chest', 'LThigh': 'Hip',
            'LToe': 'LFoot', 'LUArm': 'LShoulder', 'Neck': 'Chest',
            'RFArm': 'RUArm', 'RFoot': 'RShin', 'RHand': 'RFArm',
            'RShin': 'RThigh', 'RShoulder': 'Chest', 'RThigh': 'Hip',
            'RToe': 'RFoot', 'RUArm': 'RShoulder'}


def _build_w():
    idx = {n: i for i, n in enumerate(_JOINTS)}
    par = {idx[k]: idx[v] for k, v in _PARENTS.items()}
    adj = {j: [] for j in range(NJ)}
    for j, p in par.items():
        if j != p:
            adj[j].append(p)
            adj[p].append(j)

    def bfs_path(u, v):
        prev = {u: None}
        q = deque([u])
        while q:
            x = q.popleft()
            if x == v:
                break
            for y in adj[x]:
                if y not in prev:
                    prev[y] = x
                    q.append(y)
        path = [v]
        while prev[path[-1]] is not None:
            path.append(prev[path[-1]])
        return path[::-1]

    pairs = list(combinations(range(NJ), 2))  # 210
    c_np = np.zeros((len(pairs), NJ), np.float32)
    t_np = np.zeros((len(pairs), NJ), np.float32)
    for pi, (u, v) in enumerate(pairs):
        pa = bfs_path(u, v)
        for m in range(len(pa) - 1):
            c_np[pi, pa[m]] += 1.0 if par[pa[m]] == pa[m + 1] else -1.0
        t_np[pi, u] += 1.0
        t_np[pi, v] -= 1.0

    # W[t*63 + j*3 + c, p*3 + c] = C[p,j] (t=0) / -T[p,j] (t=1)
    eye3 = np.eye(3, dtype=np.float32)
    w_in = np.einsum('pj,cd->jcpd', c_np, eye3).reshape(63, 630)
    w_tg = np.einsum('pj,cd->jcpd', -t_np, eye3).reshape(63, 630)
    return np.ascontiguousarray(np.concatenate([w_in, w_tg], axis=0))  # [126, 630]


_W = _build_w()
# padded to 128 contraction rows, bf16 (entries are 0/+-1: exact)
_WP = np.zeros((128, 630), np.float32)
_WP[:126] = _W
_WP_BF16 = _WP.astype(ml_dtypes.bfloat16)

_NC = None


def _build_bass(n_reps=1):
    import concourse.bacc as bacc
    import concourse.mybir as mybir
    import concourse.tile as tile

    f32 = mybir.dt.float32
    bf16 = mybir.dt.bfloat16
    nc = bacc.Bacc("TRN2", target_bir_lowering=False, debug=False)

    z = nc.dram_tensor("z", [B_CORE, 128], bf16, kind="ExternalInput")
    out = nc.dram_tensor("out", [126, 1], f32, kind="ExternalOutput")

    w_dram = nc.inline_tensor(_WP_BF16, name="w_const")

    n_cols = N_GRP * N_CCH  # 40 abs-reduce results per rep

    with tile.TileContext(nc) as tc:
        with (
            tc.tile_pool(name="consts", bufs=1) as consts,
            tc.tile_pool(name="zt", bufs=8) as zt_pool,
            tc.tile_pool(name="accp", bufs=2) as accp,
            tc.tile_pool(name="psumD", bufs=4, space="PSUM") as psumD_pool,
            tc.tile_pool(name="misc", bufs=1) as misc,
        ):
            w_sb = consts.tile([128, 630], bf16)
            nc.sync.dma_start(w_sb[:], w_dram[:])

            # abs dump (never read back); bf16 to halve SBUF write traffic
            scratch = misc.tile([126, 1024], bf16)
            # prime the ACT Abs table so LoadActFuncSet overlaps the
            # initial DMA fill instead of stalling the first abs-reduce
            nc.scalar.activation(scratch[0:1, 0:1], scratch[0:1, 0:1],
                                 mybir.ActivationFunctionType.Abs)

            for rep in range(n_reps):
                acc = accp.tile([126, 2 * n_cols], f32, tag="acc")
                final = accp.tile([126, 1], f32, tag="final")

                # greedy ACT/DVE balance (ns-per-op estimates; ACT =
                # activation + accumulator read). GPSIMD cannot touch
                # PSUM on TRN2, so Pool sits this out.
                # ACT = 853 (1024 cols) + 143 psum access + 279 accum-read
                # (TRN2-measured; hw_specs' 187 is a TRN3-era compromise).
                # DVE pre-charged with the rep's final acc reduce.
                eng_t = {"act": 0.0, "dve": 210.0}
                cost = {"act": 1275.0, "dve": 1192.0}
                n_c = 0

                def emit_absred(dps):
                    nonlocal n_c
                    col = acc[:, n_c:n_c + 1]
                    n_c += 1
                    e = min(eng_t, key=lambda k: eng_t[k] + cost[k])
                    eng_t[e] += cost[e]
                    if e == "act":
                        nc.scalar.activation(
                            scratch[:], dps[:],
                            mybir.ActivationFunctionType.Abs,
                            accum_out=col)
                    else:
                        nc.vector.tensor_reduce(
                            col, dps[:],
                            axis=mybir.AxisListType.X,
                            op=mybir.AluOpType.add,
                            apply_absolute_value=True)

                for g in range(N_GRP):
                    zt = zt_pool.tile([128, G_ROWS], bf16)
                    nc.sync.dma_start_transpose(
                        zt[:], z[g * G_ROWS:(g + 1) * G_ROWS, :])

                    for c in range(N_CCH):
                        dps = psumD_pool.tile([126, 1024], f32)
                        nc.tensor.matmul(
                            dps[:, 0:512],
                            w_sb[:, c * 126:(c + 1) * 126], zt[:, 0:512])
                        nc.tensor.matmul(
                            dps[:, 512:1024],
                            w_sb[:, c * 126:(c + 1) * 126], zt[:, 512:1024])
                        emit_absred(dps)

                nc.vector.tensor_reduce(
                    final[:], acc[:, 0:n_c], axis=mybir.AxisListType.X,
                    op=mybir.AluOpType.add)
                nc.sync.dma_start(out[:], final[:])

    nc.compile()
    return nc


def kernel(input, target):
    global _NC
    from concourse.bass_utils import run_bass_kernel_spmd

    if _NC is None:
        _NC = _build_bass()

    inp = np.asarray(input, dtype=np.float32)
    tgt = np.asarray(target, dtype=np.float32)
    assert inp.shape == (B_FULL, NJ * 3) and tgt.shape == (B_FULL, NJ * 3)

    z = np.zeros((B_FULL, 128), dtype=ml_dtypes.bfloat16)
    z[:, 0:63] = inp.astype(ml_dtypes.bfloat16)
    z[:, 63:126] = tgt.astype(ml_dtypes.bfloat16)

    in_maps = []
    for i in range(N_CORES):
        sl = slice(i * B_CORE, (i + 1) * B_CORE)
        in_maps.append({"z": np.ascontiguousarray(z[sl])})

    res = run_bass_kernel_spmd(_NC, in_maps, core_ids=list(range(N_CORES)))
    total = np.float64(0.0)
    for r in res.results:
        total += np.float64(r["out"].astype(np.float64).sum())
    return np.array([total / B_FULL], dtype=np.float32)


# revision 21
# speedup vs baseline: 1.1820x; 1.0812x over previous
"""Trainium2 Bass kernel for the compositional skeleton loss.

loss = mean_b sum_{pairs p, xyz c} | (C @ bones_in)[b,p,c] - (T @ bones_tgt)[b,p,c] |

Reformulated as one matmul per batch row:  delta_row = z_row @ W, where
z_row = [input_row (63), target_row (63)] and W is [126, 630] built from the
signed path-sum matrix C and the endpoint-diff matrix T (block structure over
the 3 xyz channels), followed by abs + total sum, / B.

Device dataflow (per core, pure data parallel over batch across 8 cores):
  - host casts z to bf16 (loss rel err ~1e-6, tolerance 2e-2) padded to
    128 features
  - the DMA XBAR transpose loads z.T straight from DRAM into SBUF
    ([128 features, batch]), so the PE does no transposes and no
    PSUM->SBUF staging copies are needed
  - bf16 matmuls W_chunk.T @ zT fill [126, 1024] PSUM tiles (1 cyc/row
    vs fp32's 4)
  - abs+sum of each PSUM tile is load-balanced across ACT (activation
    with accum_out, 19 tiles) and DVE (tensor_reduce X, 21 tiles) --
    the only two engines that can read PSUM on TRN2 (GPSIMD/Pool and
    DMA cannot)
  - the ~40 per-partition partial columns go straight to DRAM; the host
    adds them up. No on-device final reduce: it would sit in one
    engine's in-order queue and head-block the next rep's abs-reduces
    behind a cross-engine barrier.

This is vector-floor-bound: 40960 PSUM columns per core split across
ACT + DVE at ~1.7 cols/ns combined, with the PE (bf16 matmuls) at ~60%
occupancy and DMA at ~25%. The 19/21 split was tuned on hardware
(18/22 and 20/20 both measure ~1us slower).
"""

import numpy as np
from collections import deque
from itertools import combinations

import ml_dtypes

# ---------------------------------------------------------------- constants
NJ = 21
B_FULL = 65536
N_CORES = 8
B_CORE = B_FULL // N_CORES  # 8192
N_GRP = 8                   # batch groups per core
G_ROWS = B_CORE // N_GRP    # 1024 rows per group
N_CCH = 5                   # 630 = 5 x 126 output-column chunks

_JOINTS = ['Ab', 'Chest', 'Head', 'Hip', 'LFArm', 'LFoot', 'LHand', 'LShin',
           'LShoulder', 'LThigh', 'LToe', 'LUArm', 'Neck', 'RFArm', 'RFoot',
           'RHand', 'RShin', 'RShoulder', 'RThigh', 'RToe', 'RUArm']
_PARENTS = {'Ab': 'Hip', 'Chest': 'Ab', 'Head': 'Neck', 'Hip': 'Hip',
            'LFArm': 'LUArm', 'LFoot': 'LShin', 'LHand': 'LFArm',
            'LShin': 'LThigh', 'LShoulder': 'Chest', 'LThigh': 'Hip',
            'LToe': 'LFoot', 'LUArm': 'LShoulder', 'Neck': 'Chest',
            'RFArm': 'RUArm', 'RFoot': 'RShin', 'RHand': 'RFArm',
            'RShin': 'RThigh', 'RShoulder': 'Chest', 'RThigh': 'Hip',
            'RToe': 'RFoot', 'RUArm': 'RShoulder'}


def _build_w():
    idx = {n: i for i, n in enumerate(_JOINTS)}
    par = {idx[k]: idx[v] for k, v in _PARENTS.items()}
    adj = {j: [] for j in range(NJ)}
    for j, p in par.items():
        if j != p:
            adj[j].append(p)
            adj[p].append(j)

    def bfs_path(u, v):
        prev = {u: None}
        q = deque([u])
        while q:
            x = q.popleft()
            if x == v:
                break
            for y in adj[x]:
                if y not in prev:
                    prev[y] = x
                    q.append(y)
        path = [v]
        while prev[path[-1]] is not None:
            path.append(prev[path[-1]])
        return path[::-1]

    pairs = list(combinations(range(NJ), 2))  # 210
    c_np = np.zeros((len(pairs), NJ), np.float32)
    t_np = np.zeros((len(pairs), NJ), np.float32)
    for pi, (u, v) in enumerate(pairs):
        pa = bfs_path(u, v)
        for m in range(len(pa) - 1):
            c_np[pi, pa[m]] += 1.0 if par[pa[m]] == pa[m + 1] else -1.0
        t_np[pi, u] += 1.0
        t_np[pi, v] -= 1.0

    # W[t*63 + j*3 + c, p*3 + c] = C[p,j] (t=0) / -T[p,j] (t=1)
    eye3 = np.eye(3, dtype=np.float32)
    w_in = np.einsum('pj,cd->jcpd', c_np, eye3).reshape(63, 630)
    w_tg = np.einsum('pj,cd->jcpd', -t_np, eye3).reshape(63, 630)
    return np.ascontiguousarray(np.concatenate([w_in, w_tg], axis=0))  # [126, 630]


_W = _build_w()
# padded to 128 contraction rows, bf16 (entries are 0/+-1: exact)
_WP = np.zeros((128, 630), np.float32)
_WP[:126] = _W
_WP_BF16 = _WP.astype(ml_dtypes.bfloat16)

_NC = None


def _build_bass(n_reps=1):
    import concourse.bacc as bacc
    import concourse.mybir as mybir
    import concourse.tile as tile

    f32 = mybir.dt.float32
    bf16 = mybir.dt.bfloat16
    nc = bacc.Bacc("TRN2", target_bir_lowering=False, debug=False)

    z = nc.dram_tensor("z", [B_CORE, 128], bf16, kind="ExternalInput")
    out = nc.dram_tensor("out", [126, 44], f32, kind="ExternalOutput")

    w_dram = nc.inline_tensor(_WP_BF16, name="w_const")

    n_cols = N_GRP * N_CCH  # 40 abs-reduce results per rep

    with tile.TileContext(nc) as tc:
        with (
            tc.tile_pool(name="consts", bufs=1) as consts,
            tc.tile_pool(name="zt", bufs=8) as zt_pool,
            tc.tile_pool(name="accp", bufs=2) as accp,
            tc.tile_pool(name="psumD", bufs=4, space="PSUM") as psumD_pool,
            tc.tile_pool(name="misc", bufs=1) as misc,
        ):
            w_sb = consts.tile([128, 630], bf16)
            nc.sync.dma_start(w_sb[:], w_dram[:])

            # abs dump (never read back); bf16 to halve SBUF write traffic
            scratch = misc.tile([126, 1024], bf16)
            # prime the ACT Abs table so LoadActFuncSet overlaps the
            # initial DMA fill instead of stalling the first abs-reduce
            nc.scalar.activation(scratch[0:1, 0:1], scratch[0:1, 0:1],
                                 mybir.ActivationFunctionType.Abs)

            for rep in range(n_reps):
                acc = accp.tile([126, 44], f32, tag="acc")

                # greedy ACT/DVE balance (ns-per-op estimates; ACT =
                # activation + accumulator read). GPSIMD cannot touch
                # PSUM on TRN2, so Pool sits this out.
                # ACT = 853 (1024 cols) + 143 psum access + 279 accum-read
                # (TRN2-measured; hw_specs' 187 is a TRN3-era compromise).
                eng_t = {"act": 0.0, "dve": 0.0}
                cost = {"act": 1275.0, "dve": 1192.0}
                n_c = 0

                def emit_absred(dps):
                    nonlocal n_c
                    col = acc[:, n_c:n_c + 1]
                    n_c += 1
                    e = min(eng_t, key=lambda k: eng_t[k] + cost[k])
                    eng_t[e] += cost[e]
                    if e == "act":
                        nc.scalar.activation(
                            scratch[:], dps[:],
                            mybir.ActivationFunctionType.Abs,
                            accum_out=col)
                    else:
                        nc.vector.tensor_reduce(
                            col, dps[:],
                            axis=mybir.AxisListType.X,
                            op=mybir.AluOpType.add,
                            apply_absolute_value=True)

                for g in range(N_GRP):
                    zt = zt_pool.tile([128, G_ROWS], bf16)
                    nc.sync.dma_start_transpose(
                        zt[:], z[g * G_ROWS:(g + 1) * G_ROWS, :])

                    for c in range(N_CCH):
                        dps = psumD_pool.tile([126, 1024], f32)
                        nc.tensor.matmul(
                            dps[:, 0:512],
                            w_sb[:, c * 126:(c + 1) * 126], zt[:, 0:512])
                        nc.tensor.matmul(
                            dps[:, 512:1024],
                            w_sb[:, c * 126:(c + 1) * 126], zt[:, 512:1024])
                        emit_absred(dps)

                # no on-device final reduce: ship the ~40 partial columns
                # and let the host add them (keeps both PSUM readers on
                # abs-reduce work and shortens the drain)
                assert n_c <= 44
                nc.sync.dma_start(out[:, 0:n_c], acc[:, 0:n_c])

    nc.compile()
    return nc


def kernel(input, target):
    global _NC
    from concourse.bass_utils import run_bass_kernel_spmd

    if _NC is None:
        _NC = _build_bass()

    inp = np.asarray(input, dtype=np.float32)
    tgt = np.asarray(target, dtype=np.float32)
    assert inp.shape == (B_FULL, NJ * 3) and tgt.shape == (B_FULL, NJ * 3)

    z = np.zeros((B_FULL, 128), dtype=ml_dtypes.bfloat16)
    z[:, 0:63] = inp.astype(ml_dtypes.bfloat16)
    z[:, 63:126] = tgt.astype(ml_dtypes.bfloat16)

    in_maps = []
    for i in range(N_CORES):
        sl = slice(i * B_CORE, (i + 1) * B_CORE)
        in_maps.append({"z": np.ascontiguousarray(z[sl])})

    res = run_bass_kernel_spmd(_NC, in_maps, core_ids=list(range(N_CORES)))
    total = np.float64(0.0)
    for r in res.results:
        total += np.float64(r["out"].astype(np.float64).sum())
    return np.array([total / B_FULL], dtype=np.float32)
